# revision 1
# baseline (speedup 1.0000x reference)
"""Causal self-attention Trainium2 Bass kernel.

Problem: B=4, T=2048, D=1024, H=16, head_dim=64.
Sharding: 8 cores = (batch b in 0..3) x (head-group g in 0..1, 8 heads each).
Each core computes a partial projection output for its batch over its 512
model dims; the host sums the two partials per batch (b_proj is fed to the
g==0 core only).

All matmuls run in float32r (TF32-like, full PE rate at N=512).
This environment has a large fixed per-instruction cost, so the kernel
batches DMAs, fuses bias adds into evacuation ops, and keeps instruction
count minimal.
"""

import numpy as np

import concourse.bacc as bacc
import concourse.bass as bass
import concourse.mybir as mybir
import concourse.tile as tile
from concourse.bass_utils import run_bass_kernel_spmd
from concourse.masks import make_identity

F32 = mybir.dt.float32
F32R = mybir.dt.float32r
AF = mybir.ActivationFunctionType

B, T, D, H = 4, 2048, 1024, 16
HD = 64              # head dim
HPC = 8              # heads per core
DC = HPC * HD        # 512 model dims per core
SCALE = 1.0 / np.sqrt(HD)

_NC_CACHE = {}


def build_nc(t=T, reps=1, phases="ABC", no_mask=False, no_norm=False,
             no_exp=False):
    """Build the single-core SPMD program. t = sequence length (for small sims).
    reps>1 repeats the computation (device-time measurement); phases/no_*
    are timing-ablation knobs (wrong numerics when used)."""
    nt = t // 128          # 128-row tiles over time
    nq = t // 512          # 512-col chunks over time
    ng = t // 1024         # 1024-col groups over time
    KC = D // 128          # 8 contraction chunks for qkv
    MQK = DC // 128        # 4 feature tiles for each of q,k

    nc = bacc.Bacc("TRN2", target_bir_lowering=False, debug=False)

    xT_d = nc.dram_tensor("xT", [D, t], F32R, kind="ExternalInput")
    wq_d = nc.dram_tensor("wq", [D, DC], F32R, kind="ExternalInput")
    wk_d = nc.dram_tensor("wk", [D, DC], F32R, kind="ExternalInput")
    wv_d = nc.dram_tensor("wv", [D, DC], F32R, kind="ExternalInput")
    bq_d = nc.dram_tensor("bq", [1, DC], F32, kind="ExternalInput")
    bk_d = nc.dram_tensor("bk", [1, DC], F32, kind="ExternalInput")
    bv_d = nc.dram_tensor("bv", [1, DC], F32, kind="ExternalInput")
    wp_d = nc.dram_tensor("wp", [DC, D], F32R, kind="ExternalInput")
    bp_d = nc.dram_tensor("bp", [1, D], F32, kind="ExternalInput")
    ones_d = nc.dram_tensor("cones", [1, 512], F32R, kind="ExternalInput")
    out_d = nc.dram_tensor("out", [t, D], F32, kind="ExternalOutput")

    with tile.TileContext(nc) as tc:
      for _rep in range(reps):
        with tc.tile_pool(name="persist", bufs=1) as persist, \
             tc.tile_pool(name="vpool", bufs=1) as vpool, \
             tc.tile_pool(name="qkpool", bufs=1) as qkpool:

            # resident qk^T: [:, m, :] = q^T feats tile m, [:, 4+m, :] = k^T
            qkTb = qkpool.tile([128, 2 * MQK, t], F32R)

            # static mask: tmask[p, u] = 1 iff u - p >= 384; slice
            # tmask[:, 384-s:512] masks a diagonal region with offset s
            tmask = persist.tile([128, 512], F32)
            nc.gpsimd.memset(tmask[:], 1.0)
            nc.gpsimd.affine_select(
                out=tmask[:], in_=tmask[:],
                compare_op=mybir.AluOpType.is_ge, fill=0.0,
                base=-384, pattern=[[1, 512]], channel_multiplier=-1)
            # broadcast constants
            ones_bc = persist.tile([128, nt * HPC], F32R)
            nc.gpsimd.dma_start(
                ones_bc[:], ones_d[0:1, 0:nt * HPC].to_broadcast([128, nt * HPC]))
            bv_bc = persist.tile([128, DC], F32)
            nc.gpsimd.dma_start(bv_bc[:], bv_d[0:1, :].to_broadcast([128, DC]))
            bp_bc = persist.tile([128, D], F32)
            nc.gpsimd.dma_start(bp_bc[:], bp_d[0:1, :].to_broadcast([128, D]))
            # partition-major per-feature-tile bias columns [128, MQK]
            bqp = persist.tile([128, MQK], F32)
            nc.sync.dma_start(bqp[:], bq_d.rearrange("o (m p) -> p (o m)", p=128))
            bkp = persist.tile([128, MQK], F32)
            nc.sync.dma_start(bkp[:], bk_d.rearrange("o (m p) -> p (o m)", p=128))

            # v' mega-tile: [128, nt, 8*65]; col h*65+64 holds ones
            vpm = vpool.tile([128, nt, HPC * (HD + 1)], F32R)
            nc.vector.tensor_copy(
                vpm.rearrange("p t (h e) -> p (t h) e", e=HD + 1)[:, :, HD:HD + 1],
                ones_bc[:].unsqueeze(2))

            # ---------------- Phase A: qkv ----------------
            with tc.tile_pool(name="phA_sb", bufs=1) as pa, \
                 tc.tile_pool(name="phA_w", bufs=2) as pw, \
                 tc.tile_pool(name="phA_ps", bufs=2, space="PSUM") as pps:

                # x^T resident: one tile [128, KC, t], single DMA
                xTb = pa.tile([128, KC, t], F32R)
                nc.sync.dma_start(
                    xTb[:], xT_d.rearrange("(k p) t -> p k t", p=128))

                # q^T / k^T -> psum -> (bias-add) resident qkTb
                for sec, (w_d, b_s) in enumerate(
                        ((wq_d, bqp), (wk_d, bkp)) if ("q" in phases or "A" in phases) else ()):
                    ws = pw.tile([128, KC, DC], F32R, name=f"ws{sec}", tag="wsec")
                    nc.sync.dma_start(ws[:], w_d.rearrange("(k p) c -> p k c", p=128))
                    for m in range(MQK):
                        for np2 in range(nq // 2):
                            ps = pps.tile([128, 1024], F32, name="qkps", tag="psqk")
                            for k in range(KC):
                                for half in range(2):
                                    n = 2 * np2 + half
                                    nc.tensor.matmul(
                                        ps[:, half * 512:(half + 1) * 512],
                                        ws[:, k, m * 128:(m + 1) * 128],
                                        xTb[:, k, n * 512:(n + 1) * 512],
                                        start=(k == 0), stop=(k == KC - 1))
                            nc.vector.tensor_scalar_add(
                                qkTb[:, sec * MQK + m,
                                     np2 * 1024:(np2 + 1) * 1024],
                                ps[:], b_s[:, m:m + 1])

                # v natural (+bias) -> strided copy into v' tiles
                if "v" in phases or "A" in phases:
                    wvs = pw.tile([128, KC, DC], F32R, name="wvs", tag="wsec")
                    nc.sync.dma_start(wvs[:], wv_d.rearrange("(k p) c -> p k c", p=128))
                    for tt in range(nt):
                        ps = pps.tile([128, 512], F32, name="vps", tag="psv")
                        for k in range(KC):
                            nc.tensor.matmul(
                                ps[:],
                                xTb[:, k, tt * 128:(tt + 1) * 128],
                                wvs[:, k, :],
                                start=(k == 0), stop=(k == KC - 1))
                        nc.vector.tensor_add(
                            vpm[:, tt].rearrange("p (h e) -> p h e", e=HD + 1)[:, :, 0:HD],
                            ps.rearrange("p (h e) -> p h e", e=HD),
                            bv_bc.rearrange("p (h e) -> p h e", e=HD))

            # ---------------- Phase B: attention ----------------
            if "B" not in phases:
                continue
            with tc.tile_pool(name="yT", bufs=1) as ypool:
                yT = [ypool.tile([128, t], F32R, name=f"yT{f}", tag=f"yT{f}")
                      for f in range(MQK)]

                with tc.tile_pool(name="esb", bufs=2) as pesb, \
                     tc.tile_pool(name="norm", bufs=1) as pnorm, \
                     tc.tile_pool(name="sc_ps", bufs=1, space="PSUM") as pscps, \
                     tc.tile_pool(name="y_ps", bufs=1, space="PSUM") as pyps:

                    for f in range(MQK):
                        for hh in range(2):
                            h = 2 * f + hh
                            qh = qkTb[:, f][hh * HD:(hh + 1) * HD, :]
                            kh = qkTb[:, MQK + f][hh * HD:(hh + 1) * HD, :]
                            y_acc = pyps.tile([HD + 1, t], F32,
                                              name=f"yacc{h}", tag="yacc")
                            for kc in range(nt):
                                nmin = kc // 4
                                dn = kc // 4      # diagonal 512-chunk index
                                s = 128 * (kc % 4)
                                dlo = dn * 512 + s   # first live column
                                sp = pscps.tile([128, t], F32,
                                                name="scps", tag="scps")
                                for n in range(nmin, nq):
                                    w0 = dlo if n == dn else n * 512
                                    nc.tensor.matmul(
                                        sp[:, w0:(n + 1) * 512],
                                        kh[:, kc * 128:(kc + 1) * 128],
                                        qh[:, w0:(n + 1) * 512],
                                        start=True, stop=True)
                                esb = pesb.tile([128, t], F32R,
                                                name="esb", tag="esb")
                                nc.scalar.activation(esb[:, dlo:], sp[:, dlo:],
                                                     AF.Copy if no_exp else AF.Exp,
                                                     scale=float(SCALE))
                                if not no_mask:
                                    nc.vector.tensor_mul(
                                        esb[:, dlo:dlo + 128],
                                        esb[:, dlo:dlo + 128],
                                        tmask[:, 384:512])
                                for n in range(nmin, nq):
                                    w0 = dlo if n == dn else n * 512
                                    nc.tensor.matmul(
                                        y_acc[:, w0:(n + 1) * 512],
                                        vpm[:, kc, h * (HD + 1):(h + 1) * (HD + 1)],
                                        esb[:, w0:(n + 1) * 512],
                                        start=(kc == 0), stop=(kc == 4 * n + 3))
                            # normalize: yT[f][hh*64:, :] = y/denom
                            if no_norm:
                                nc.vector.tensor_copy(
                                    yT[f][hh * HD:(hh + 1) * HD, :],
                                    y_acc[0:HD, :])
                            else:
                                rec = pnorm.tile([1, t], F32, name="rec", tag="rec")
                                nc.vector.reciprocal(rec[:], y_acc[HD:HD + 1, :])
                                rb = pnorm.tile([HD, t], F32, name="rb", tag="rb")
                                nc.gpsimd.partition_broadcast(rb[:], rec[:])
                                nc.vector.tensor_mul(
                                    yT[f][hh * HD:(hh + 1) * HD, :],
                                    y_acc[0:HD, :], rb[:])

                # ---------------- Phase C: projection ----------------
                if "C" not in phases:
                    continue
                with tc.tile_pool(name="phC_sb", bufs=1) as pc, \
                     tc.tile_pool(name="phC_evac", bufs=3) as pcev, \
                     tc.tile_pool(name="phC_ps", bufs=3, space="PSUM") as pcps:
                    wpb = pc.tile([128, MQK, D], F32R)
                    nc.sync.dma_start(
                        wpb[:], wp_d.rearrange("(m p) o -> p m o", p=128))
                    for qtp in range(nt // 2):
                        ev = pcev.tile([128, 2, D], F32, name="prev", tag="prev")
                        for half in range(2):
                            qt = 2 * qtp + half
                            ps = pcps.tile([128, 1024], F32, name="prps", tag="prps")
                            for oc in range(D // 512):
                                for m in range(MQK):
                                    nc.tensor.matmul(
                                        ps[:, oc * 512:(oc + 1) * 512],
                                        yT[m][:, qt * 128:(qt + 1) * 128],
                                        wpb[:, m, oc * 512:(oc + 1) * 512],
                                        start=(m == 0), stop=(m == MQK - 1))
                            nc.vector.tensor_add(ev[:, half, :], ps[:], bp_bc[:])
                        nc.sync.dma_start(
                            out_d[qtp * 256:(qtp + 1) * 256, :]
                            .rearrange("(a p) o -> p a o", p=128),
                            ev[:])

    nc.finalize()
    return nc


def make_in_maps(x, w_attn, b_attn, w_proj, b_proj):
    x = np.ascontiguousarray(np.asarray(x, dtype=np.float32))
    w_attn = np.asarray(w_attn, dtype=np.float32)
    b_attn = np.asarray(b_attn, dtype=np.float32)
    w_proj = np.asarray(w_proj, dtype=np.float32)
    b_proj = np.asarray(b_proj, dtype=np.float32)
    in_maps = []
    for c in range(8):
        b, g = c // 2, c % 2
        sl = slice(DC * g, DC * (g + 1))
        in_maps.append({
            "xT": np.ascontiguousarray(x[b].T),
            "wq": np.ascontiguousarray(w_attn[:, 0 * D:][:, sl]),
            "wk": np.ascontiguousarray(w_attn[:, 1 * D:][:, sl]),
            "wv": np.ascontiguousarray(w_attn[:, 2 * D:][:, sl]),
            "bq": np.ascontiguousarray(b_attn[0 * D:1 * D][sl][None, :]),
            "bk": np.ascontiguousarray(b_attn[1 * D:2 * D][sl][None, :]),
            "bv": np.ascontiguousarray(b_attn[2 * D:3 * D][sl][None, :]),
            "wp": np.ascontiguousarray(w_proj[sl, :]),
            "bp": np.ascontiguousarray(
                (b_proj if g == 0 else np.zeros_like(b_proj))[None, :]),
            "cones": np.ones((1, 512), dtype=np.float32),
        })
    return in_maps


def kernel(x, w_attn, b_attn, w_proj, b_proj, _trace=False, _trace_kwargs=None):
    if "nc" not in _NC_CACHE:
        _NC_CACHE["nc"] = build_nc()
    nc = _NC_CACHE["nc"]
    in_maps = make_in_maps(x, w_attn, b_attn, w_proj, b_proj)
    kw = {}
    if _trace:
        kw["trace"] = True
        if _trace_kwargs:
            kw.update(_trace_kwargs)
    res = run_bass_kernel_spmd(nc, in_maps, core_ids=list(range(8)), **kw)
    outs = [res.results[c]["out"] for c in range(8)]
    out = np.empty((B, T, D), dtype=np.float32)
    for b in range(B):
        np.add(outs[2 * b], outs[2 * b + 1], out=out[b])
    kernel._last_results = res
    return out


if __name__ == "__main__":
    nc = build_nc()
    print("built ok")



# revision 2
# speedup vs baseline: 8.1673x; 8.1673x over previous
"""Causal self-attention Trainium2 Bass kernel — static-instruction-minimal.

Empirically this environment charges ~65us per STATIC instruction in the
NEFF (load/dispatch dominated), while executed instructions, matmul width,
and DMA descriptor counts are ~free.  The kernel therefore wraps all work
in dynamic For_i loops so the static program is ~400 instructions instead
of the ~2100 of a fully unrolled kernel.

Key devices to keep every loop-varying operand off the matmul stationary
port (walrus forbids register offsets in ldweights):
  - stationary tiles are staged into fixed scratch slots by DMA/DVE copies
    with dynamic source offsets;
  - PSUM accumulation groups are opened/closed by zero-stationary matmuls
    so start/stop flags stay static inside loops;
  - q/k and y are stored in head-major duplicated layouts ([64, slot, t])
    so all head/chunk addressing is affine in loop registers;
  - scores are computed full-width (no causal windowing — flops are free)
    and masked post-exp with a sliced static triangular mask;
  - the projection is computed transposed (out^T) so weights are the
    stationary; the host transposes back.

Sharding: 8 cores = (batch b in 0..3) x (head-group g in 0..1, 8 heads).
Core (b, g) computes a partial projection output for batch b over its 512
model dims; the host sums the two partials per batch (b_proj folded into
the g==0 core only).  Matmul inputs are shipped as bf16 (tolerance 2e-2).
"""

import numpy as np
import ml_dtypes

import concourse.bacc as bacc
import concourse.mybir as mybir
import concourse.tile as tile
from concourse.bass import ds
from concourse.bass_utils import run_bass_kernel_spmd

F32 = mybir.dt.float32
BF16 = mybir.dt.bfloat16
AF = mybir.ActivationFunctionType
BF = ml_dtypes.bfloat16

B, T, D, H = 4, 2048, 1024, 16
HD = 64              # head dim
HPC = 8              # heads per core
DC = HPC * HD        # 512 model dims per core
SCALE = 1.0 / np.sqrt(HD)

_NC_CACHE = {}


def build_nc(t=T, reps=1):
    """Build the single-core SPMD program. t must be T for real runs."""
    nt = t // 128
    KC = D // 128          # 8 contraction chunks over model dim

    nc = bacc.Bacc("TRN2", target_bir_lowering=False, debug=False)

    xT_d = nc.dram_tensor("xT", [D, t], BF16, kind="ExternalInput")
    # weight matrices carry the bias as an extra contraction row (vs ones)
    wqkv_d = nc.dram_tensor("wqkv", [D + 1, 3 * DC], BF16, kind="ExternalInput")
    wp_d = nc.dram_tensor("wp", [DC + 1, D], BF16, kind="ExternalInput")
    outT_d = nc.dram_tensor("outT", [D, t], F32, kind="ExternalOutput")

    NM = 2 * DC // 128     # 8 output chunks of 128 feats (q, k)

    with tile.TileContext(nc) as tc:
      for _rep in range(reps):
        with tc.tile_pool(name="ptop", bufs=1) as ptop:
          # normalized y^T, head-major: slot h rows 0:64 = head h dims
          ydup = ptop.tile([64, HPC, t], BF16)

          with tc.tile_pool(name="pab", bufs=1) as pab:
            # q/k head-major dup: slots 0-7 q^T, 8-15 k^T
            qkvdup = pab.tile([64, 2 * HPC, t], BF16)
            # v natural + ones col: [t-tile][head*(HD+1)]
            vpm = pab.tile([128, nt, HPC * (HD + 1)], BF16, name="vpm", tag="vpm")
            # causal mask: Tm[p, u] = 1 iff u - t - p >= 0; slice
            # Tm[:, t - 128*kc :][:, :t] masks keys chunk kc vs all queries
            Tm = pab.tile([128, 2 * t], BF16, name="Tm", tag="Tm")
            zeros = pab.tile([128, 128], BF16, name="zeros", tag="zeros")
            onesr = pab.tile([1, t], BF16, name="onesr", tag="onesr")

            nc.gpsimd.memset(vpm[:], 1.0)
            nc.gpsimd.memset(onesr[:], 1.0)
            nc.gpsimd.memset(zeros[:], 0.0)
            nc.gpsimd.memset(Tm[:], 1.0)
            nc.gpsimd.affine_select(
                out=Tm[:], in_=Tm[:], compare_op=mybir.AluOpType.is_ge,
                fill=0.0, base=-t, pattern=[[1, 2 * t]], channel_multiplier=-1)

            # ---------------- Phase A: qkv ----------------
            with tc.tile_pool(name="pa", bufs=1) as pa, \
                 tc.tile_pool(name="pa_ps", bufs=1, space="PSUM") as paps:
                xTb = pa.tile([128, KC, t], BF16)
                ws = pa.tile([128, 128], BF16, name="ws", tag="ws")
                nc.sync.dma_start(xTb[:], xT_d.rearrange("(k p) t -> p k t", p=128))

                psA = paps.tile([128, t], F32, name="psA", tag="psA")
                with tc.For_i(0, NM) as m:
                    for k in range(KC):
                        nc.sync.dma_start(
                            ws[:], wqkv_d[k * 128:(k + 1) * 128, ds(m * 128, 128)])
                        for w in range(t // 512):
                            nc.tensor.matmul(
                                psA[:, w * 512:(w + 1) * 512],
                                ws[:], xTb[:, k, w * 512:(w + 1) * 512],
                                start=(k == 0), stop=False)
                    nc.sync.dma_start(
                        ws[0:1, :], wqkv_d[D:D + 1, ds(m * 128, 128)])
                    for w in range(t // 512):
                        nc.tensor.matmul(
                            psA[:, w * 512:(w + 1) * 512],
                            ws[0:1, :], onesr[:, w * 512:(w + 1) * 512],
                            start=False, stop=True)
                    nc.vector.tensor_copy(
                        qkvdup[:, ds(m * 2, 1), :], psA[0:64, :].unsqueeze(1))
                    nc.vector.tensor_copy(
                        qkvdup[:, ds(m * 2 + 1, 1), :], psA[64:128, :].unsqueeze(1))

                # v natural: x-chunk stationary, wv moving
                wvb = pa.tile([128, KC, DC], BF16, name="wvb", tag="wvb")
                nc.sync.dma_start(
                    wvb[:], wqkv_d[0:D, 2 * DC:3 * DC]
                    .rearrange("(k p) c -> p k c", p=128))
                wvbias = pa.tile([1, DC], BF16, name="wvbias", tag="wvbias")
                nc.sync.dma_start(wvbias[:], wqkv_d[D:D + 1, 2 * DC:3 * DC])
                xts = pa.tile([128, 128], BF16, name="xts", tag="xts")
                psv = paps.tile([128, DC], F32, name="psv", tag="psv")
                with tc.For_i(0, nt) as tt:
                    for k in range(KC):
                        nc.vector.tensor_copy(
                            xts[:].unsqueeze(1),
                            xTb[:, k, ds(tt * 128, 128)].unsqueeze(1))
                        nc.tensor.matmul(psv[:], xts[:], wvb[:, k, :],
                                         start=(k == 0), stop=False)
                    nc.tensor.matmul(psv[:], onesr[0:1, 0:128], wvbias[:],
                                     start=False, stop=True)
                    nc.vector.tensor_copy(
                        vpm[:, ds(tt, 1), :]
                        .rearrange("p a (h e) -> p a h e", e=HD + 1)[:, :, :, 0:HD],
                        psv[:].rearrange("p (h e) -> p h e", e=HD).unsqueeze(1))

            # ---------------- Phase B: attention ----------------
            with tc.tile_pool(name="pb", bufs=1) as pb, \
                 tc.tile_pool(name="pb_ps", bufs=1, space="PSUM") as pbps:
                khs = pb.tile([64, 128], BF16, name="khs", tag="khs")
                vhs = pb.tile([128, HD + 1], BF16, name="vhs", tag="vhs")
                esb = pb.tile([128, t], BF16, name="esb", tag="esb")
                rec = pb.tile([1, t], F32, name="rec", tag="rec")
                rb = pb.tile([64, t], F32, name="rb", tag="rb")
                sp = pbps.tile([128, t], F32, name="sp", tag="sp")
                yacc = pbps.tile([HD + 1, t], F32, name="yacc", tag="yacc")
                with tc.For_i(0, HPC) as h:
                    for w in range(t // 512):
                        nc.tensor.matmul(
                            yacc[:, w * 512:(w + 1) * 512],
                            zeros[:, 0:HD + 1], Tm[:, 0:512],
                            start=True, stop=False)
                    with tc.For_i(0, nt) as kc:
                        nc.vector.tensor_copy(
                            khs[:].unsqueeze(1),
                            qkvdup[:, ds(HPC + h, 1), ds(kc * 128, 128)])
                        for w in range(t // 512):
                            nc.tensor.matmul(
                                sp[:, w * 512:(w + 1) * 512],
                                khs[:],
                                qkvdup[:, ds(h, 1), w * 512:(w + 1) * 512],
                                start=True, stop=True)
                        nc.scalar.activation(esb[:], sp[:], AF.Exp,
                                             scale=float(SCALE))
                        nc.vector.tensor_mul(esb[:], esb[:],
                                             Tm[:, ds(t - kc * 128, t)])
                        nc.vector.tensor_copy(
                            vhs[:].unsqueeze(1),
                            vpm[:, ds(kc, 1), ds(h * (HD + 1), HD + 1)])
                        for w in range(t // 512):
                            nc.tensor.matmul(
                                yacc[:, w * 512:(w + 1) * 512],
                                vhs[:], esb[:, w * 512:(w + 1) * 512],
                                start=False, stop=False)
                    for w in range(t // 512):
                        nc.tensor.matmul(
                            yacc[:, w * 512:(w + 1) * 512],
                            zeros[:, 0:HD + 1], Tm[:, 0:512],
                            start=False, stop=True)
                    nc.vector.reciprocal(rec[:], yacc[HD:HD + 1, :])
                    nc.gpsimd.partition_broadcast(rb[:], rec[:])
                    nc.vector.tensor_mul(
                        ydup[:, ds(h, 1), :],
                        yacc[0:HD, :].unsqueeze(1), rb[:].unsqueeze(1))

          # ---------------- Phase C: projection (transposed) ----------------
          with tc.tile_pool(name="pc", bufs=1) as pc, \
               tc.tile_pool(name="pc_ps", bufs=1, space="PSUM") as pcps:
            osb = pc.tile([128, D // 128, t], F32, name="osb", tag="osb")
            wps = pc.tile([64, 128], BF16, name="wps", tag="wps")
            onesc = pc.tile([1, t], BF16, name="onesc", tag="onesc")
            nc.gpsimd.memset(onesc[:], 1.0)
            pso = pcps.tile([128, t], F32, name="pso", tag="pso")
            with tc.For_i(0, D // 128) as oc:
                for h in range(HPC):
                    nc.sync.dma_start(
                        wps[:], wp_d[h * HD:(h + 1) * HD, ds(oc * 128, 128)])
                    for w in range(t // 512):
                        nc.tensor.matmul(
                            pso[:, w * 512:(w + 1) * 512],
                            wps[:], ydup[:, h, w * 512:(w + 1) * 512],
                            start=(h == 0), stop=False)
                nc.sync.dma_start(
                    wps[0:1, :], wp_d[DC:DC + 1, ds(oc * 128, 128)])
                for w in range(t // 512):
                    nc.tensor.matmul(
                        pso[:, w * 512:(w + 1) * 512],
                        wps[0:1, :], onesc[:, w * 512:(w + 1) * 512],
                        start=False, stop=True)
                nc.vector.tensor_copy(
                    osb[:, ds(oc, 1), :], pso[:].unsqueeze(1))
            nc.sync.dma_start(outT_d.rearrange("(c p) t -> p c t", p=128), osb[:])

    nc.finalize()
    return nc


def make_in_maps(x, w_attn, b_attn, w_proj, b_proj):
    x = np.asarray(x, dtype=np.float32)
    w_attn = np.asarray(w_attn, dtype=np.float32)
    b_attn = np.asarray(b_attn, dtype=np.float32)
    w_proj = np.asarray(w_proj, dtype=np.float32)
    b_proj = np.asarray(b_proj, dtype=np.float32)
    in_maps = []
    for c in range(8):
        b, g = c // 2, c % 2
        sl = slice(DC * g, DC * (g + 1))
        wqkv = np.concatenate(
            [w_attn[:, 0 * D:1 * D][:, sl],
             w_attn[:, 1 * D:2 * D][:, sl],
             w_attn[:, 2 * D:3 * D][:, sl]], axis=1)
        bqkv = np.concatenate(
            [b_attn[0 * D:1 * D][sl], b_attn[1 * D:2 * D][sl],
             b_attn[2 * D:3 * D][sl]])[None, :]
        bp = (b_proj if g == 0 else np.zeros_like(b_proj))[None, :]
        in_maps.append({
            "xT": np.ascontiguousarray(x[b].T).astype(BF),
            "wqkv": np.ascontiguousarray(
                np.concatenate([wqkv, bqkv], axis=0)).astype(BF),
            "wp": np.ascontiguousarray(
                np.concatenate([w_proj[sl, :], bp], axis=0)).astype(BF),
        })
    return in_maps


def kernel(x, w_attn, b_attn, w_proj, b_proj, _trace=False, _trace_kwargs=None):
    if "nc" not in _NC_CACHE:
        _NC_CACHE["nc"] = build_nc()
    nc = _NC_CACHE["nc"]
    in_maps = make_in_maps(x, w_attn, b_attn, w_proj, b_proj)
    kw = {}
    if _trace:
        kw["trace"] = True
        if _trace_kwargs:
            kw.update(_trace_kwargs)
    res = run_bass_kernel_spmd(nc, in_maps, core_ids=list(range(8)), **kw)
    outs = [res.results[c]["outT"] for c in range(8)]
    out = np.empty((B, T, D), dtype=np.float32)
    for b in range(B):
        np.add(outs[2 * b].T, outs[2 * b + 1].T, out=out[b])
    kernel._last_results = res
    return out


if __name__ == "__main__":
    nc = build_nc()
    fn = nc.m.functions[0]
    n = sum(len(blk.instructions) for blk in fn.blocks)
    print(f"built ok, static instructions: {n}")


# revision 5
# speedup vs baseline: 23.8335x; 2.9182x over previous
"""Causal self-attention Trainium2 Bass kernel — static-instruction-minimal.

Empirically this environment charges ~65us per STATIC instruction in the
NEFF (load/dispatch dominated), while executed instructions, matmul width,
and DMA descriptor counts are ~free.  The kernel therefore wraps all work
in dynamic For_i loops so the static program is ~400 instructions instead
of the ~2100 of a fully unrolled kernel.

Key devices to keep every loop-varying operand off the matmul stationary
port (walrus forbids register offsets in ldweights):
  - stationary tiles are staged into fixed scratch slots by DMA/DVE copies
    with dynamic source offsets;
  - PSUM accumulation groups are opened/closed by zero-stationary matmuls
    so start/stop flags stay static inside loops;
  - q/k and y are stored in head-major duplicated layouts ([64, slot, t])
    so all head/chunk addressing is affine in loop registers;
  - scores are computed full-width (no causal windowing — flops are free)
    and masked post-exp with a sliced static triangular mask;
  - the projection is computed transposed (out^T) so weights are the
    stationary; the host transposes back.

Sharding: 8 cores = (batch b in 0..3) x (head-group g in 0..1, 8 heads).
Core (b, g) computes a partial projection output for batch b over its 512
model dims; the host sums the two partials per batch (b_proj folded into
the g==0 core only).  Matmul inputs are shipped as bf16 (tolerance 2e-2).
"""

import numpy as np
import ml_dtypes

import concourse.bacc as bacc
import concourse.mybir as mybir
import concourse.tile as tile
from concourse.bass import ds
from concourse.bass_utils import run_bass_kernel_spmd

F32 = mybir.dt.float32
BF16 = mybir.dt.bfloat16
AF = mybir.ActivationFunctionType
BF = ml_dtypes.bfloat16

B, T, D, H = 4, 2048, 1024, 16
HD = 64              # head dim
HPC = 8              # heads per core
DC = HPC * HD        # 512 model dims per core
SCALE = 1.0 / np.sqrt(HD)

_NC_CACHE = {}


def build_nc(t=T, reps=1, with_bias=False):
    """Build the single-core SPMD program. t must be T for real runs.
    with_bias adds the ones x bias-row matmuls (spec fills biases with
    zeros, so the default program omits them)."""
    nt = t // 128
    KC = D // 128          # 8 contraction chunks over model dim

    nc = bacc.Bacc("TRN2", target_bir_lowering=False, debug=False)

    xT_d = nc.dram_tensor("xT", [D, t], BF16, kind="ExternalInput")
    # weight matrices carry the bias as an extra contraction row (vs ones)
    wqkv_d = nc.dram_tensor("wqkv", [D + 1, 3 * DC], BF16, kind="ExternalInput")
    wp_d = nc.dram_tensor("wp", [DC + 1, D], BF16, kind="ExternalInput")
    outT_d = nc.dram_tensor("outT", [D, t], F32, kind="ExternalOutput")

    NM = 3 * DC // 128     # 12 output chunks of 128 feats (q, k, v)

    with tile.TileContext(nc) as tc:
      for _rep in range(reps):
        with tc.tile_pool(name="ptop", bufs=1) as ptop:
          # normalized y^T, head-major: slot h rows 0:64 = head h dims
          ydup = ptop.tile([64, HPC, t], BF16)

          with tc.tile_pool(name="pab", bufs=1) as pab:
            # q/k/v head-major dup: slots 0-7 q^T, 8-15 k^T, 16-23 v^T
            qkvdup = pab.tile([64, 3 * HPC, t], BF16)
            # v natural, per head contiguous: vnat[p, h, tt, d] = v[tt*128+p, h, d]
            vnat = pab.tile([128, HPC, nt, HD], BF16, name="vnat", tag="vnat")
            # causal mask: Tm[p, u] = 1 iff u - t - p >= 0; slice
            # Tm[:, t - 128*kc :][:, :t] masks keys chunk kc vs all queries
            Tm = pab.tile([128, 2 * t], BF16, name="Tm", tag="Tm")
            zeros = pab.tile([128, 128], BF16, name="zeros", tag="zeros")
            onesr = pab.tile([1, t], BF16, name="onesr", tag="onesr")

            nc.gpsimd.memset(onesr[:], 1.0)
            nc.gpsimd.memset(zeros[:], 0.0)
            nc.gpsimd.memset(Tm[:], 1.0)
            nc.gpsimd.affine_select(
                out=Tm[:], in_=Tm[:], compare_op=mybir.AluOpType.is_ge,
                fill=0.0, base=-t, pattern=[[1, 2 * t]], channel_multiplier=-1)

            # ---------------- Phase A: qkv ----------------
            with tc.tile_pool(name="pa", bufs=1) as pa, \
                 tc.tile_pool(name="pa_ps", bufs=1, space="PSUM") as paps:
                xTb = pa.tile([128, KC, t], BF16)
                ws = pa.tile([128, KC, 128], BF16, name="ws", tag="ws")
                wsb = pa.tile([1, 128], BF16, name="wsb", tag="wsb")
                nc.sync.dma_start(xTb[:], xT_d.rearrange("(k p) t -> p k t", p=128))

                psA = paps.tile([128, t], F32, name="psA", tag="psA")
                with tc.For_i(0, NM) as m:
                    nc.sync.dma_start(
                        ws[:], wqkv_d[0:D, ds(m * 128, 128)]
                        .rearrange("(k p) c -> p k c", p=128))
                    if with_bias:
                        nc.sync.dma_start(
                            wsb[:], wqkv_d[D:D + 1, ds(m * 128, 128)])
                    for k in range(KC):
                        for w in range(t // 512):
                            nc.tensor.matmul(
                                psA[:, w * 512:(w + 1) * 512],
                                ws[:, k, :], xTb[:, k, w * 512:(w + 1) * 512],
                                start=(k == 0),
                                stop=(not with_bias and k == KC - 1))
                    if with_bias:
                        for w in range(t // 512):
                            nc.tensor.matmul(
                                psA[:, w * 512:(w + 1) * 512],
                                wsb[:], onesr[:, w * 512:(w + 1) * 512],
                                start=False, stop=True)
                    nc.vector.tensor_copy(
                        qkvdup[:, ds(m * 2, 1), :], psA[0:64, :].unsqueeze(1))
                    nc.vector.tensor_copy(
                        qkvdup[:, ds(m * 2 + 1, 1), :], psA[64:128, :].unsqueeze(1))

                # v^T slots -> v natural via DMA transpose (contiguous dst;
                # HW writes dst[p, tt, d] = src[d, tt*128 + p])
                for h in range(HPC):
                    nc.sync.dma_start(
                        vnat[:, h, :, :], qkvdup[:, 2 * HPC + h, :],
                        transpose=True)

            # ---------------- Phase B: attention ----------------
            with tc.tile_pool(name="pb", bufs=1) as pb, \
                 tc.tile_pool(name="pb_ps", bufs=1, space="PSUM") as pbps:
                khs = pb.tile([64, 128], BF16, name="khs", tag="khs")
                # vhs col HD stays 1.0 (softmax denominator ones column)
                vhs = pb.tile([128, HD + 1], BF16, name="vhs", tag="vhs")
                nc.gpsimd.memset(vhs[:], 1.0)
                esb = pb.tile([128, t], BF16, name="esb", tag="esb")
                rec = pb.tile([1, t], F32, name="rec", tag="rec")
                rb = pb.tile([64, t], F32, name="rb", tag="rb")
                sp = pbps.tile([128, t], F32, name="sp", tag="sp")
                yacc = pbps.tile([HD + 1, t], F32, name="yacc", tag="yacc")
                with tc.For_i(0, HPC) as h:
                    nc.vector.memset(yacc[:], 0.0)
                    with tc.For_i(0, nt) as kc:
                        nc.vector.tensor_copy(
                            khs[:].unsqueeze(1),
                            qkvdup[:, ds(HPC + h, 1), ds(kc * 128, 128)])
                        for w in range(t // 512):
                            nc.tensor.matmul(
                                sp[:, w * 512:(w + 1) * 512],
                                khs[:],
                                qkvdup[:, ds(h, 1), w * 512:(w + 1) * 512],
                                start=True, stop=True)
                        nc.scalar.activation(esb[:], sp[:], AF.Exp,
                                             scale=float(SCALE))
                        nc.vector.tensor_mul(esb[:], esb[:],
                                             Tm[:, ds(t - kc * 128, t)])
                        nc.vector.tensor_copy(
                            vhs[:, 0:HD].unsqueeze(1),
                            vnat[:, ds(h, 1), ds(kc, 1), :]
                            .rearrange("p a b e -> p (a b) e"))
                        for w in range(t // 512):
                            nc.tensor.matmul(
                                yacc[:, w * 512:(w + 1) * 512],
                                vhs[:], esb[:, w * 512:(w + 1) * 512],
                                start=False, stop=False,
                                skip_group_check=True)
                    nc.vector.reciprocal(rec[:], yacc[HD:HD + 1, :])
                    nc.gpsimd.partition_broadcast(rb[:], rec[:])
                    nc.vector.tensor_mul(
                        ydup[:, ds(h, 1), :],
                        yacc[0:HD, :].unsqueeze(1), rb[:].unsqueeze(1))

          # ---------------- Phase C: projection (transposed) ----------------
          with tc.tile_pool(name="pc", bufs=1) as pc, \
               tc.tile_pool(name="pc_ps", bufs=1, space="PSUM") as pcps:
            osb = pc.tile([128, D // 128, t], F32, name="osb", tag="osb")
            wps = pc.tile([64, HPC, 128], BF16, name="wps", tag="wps")
            wpsb = pc.tile([1, 128], BF16, name="wpsb", tag="wpsb")
            onesc = pc.tile([1, t], BF16, name="onesc", tag="onesc")
            nc.gpsimd.memset(onesc[:], 1.0)
            pso = pcps.tile([128, t], F32, name="pso", tag="pso")
            with tc.For_i(0, D // 128) as oc:
                nc.sync.dma_start(
                    wps[:], wp_d[0:DC, ds(oc * 128, 128)]
                    .rearrange("(h p) c -> p h c", p=64))
                if with_bias:
                    nc.sync.dma_start(
                        wpsb[:], wp_d[DC:DC + 1, ds(oc * 128, 128)])
                for h in range(HPC):
                    for w in range(t // 512):
                        nc.tensor.matmul(
                            pso[:, w * 512:(w + 1) * 512],
                            wps[:, h, :], ydup[:, h, w * 512:(w + 1) * 512],
                            start=(h == 0),
                            stop=(not with_bias and h == HPC - 1))
                if with_bias:
                    for w in range(t // 512):
                        nc.tensor.matmul(
                            pso[:, w * 512:(w + 1) * 512],
                            wpsb[:], onesc[:, w * 512:(w + 1) * 512],
                            start=False, stop=True)
                nc.vector.tensor_copy(
                    osb[:, ds(oc, 1), :], pso[:].unsqueeze(1))
            nc.sync.dma_start(outT_d.rearrange("(c p) t -> p c t", p=128), osb[:])

    nc.finalize()
    return nc


def make_in_maps(x, w_attn, b_attn, w_proj, b_proj):
    x = np.asarray(x, dtype=np.float32)
    w_attn = np.asarray(w_attn, dtype=np.float32)
    b_attn = np.asarray(b_attn, dtype=np.float32)
    w_proj = np.asarray(w_proj, dtype=np.float32)
    b_proj = np.asarray(b_proj, dtype=np.float32)
    in_maps = []
    for c in range(8):
        b, g = c // 2, c % 2
        sl = slice(DC * g, DC * (g + 1))
        wqkv = np.concatenate(
            [w_attn[:, 0 * D:1 * D][:, sl],
             w_attn[:, 1 * D:2 * D][:, sl],
             w_attn[:, 2 * D:3 * D][:, sl]], axis=1)
        bqkv = np.concatenate(
            [b_attn[0 * D:1 * D][sl], b_attn[1 * D:2 * D][sl],
             b_attn[2 * D:3 * D][sl]])[None, :]
        bp = (b_proj if g == 0 else np.zeros_like(b_proj))[None, :]
        in_maps.append({
            "xT": np.ascontiguousarray(x[b].T).astype(BF),
            "wqkv": np.ascontiguousarray(
                np.concatenate([wqkv, bqkv], axis=0)).astype(BF),
            "wp": np.ascontiguousarray(
                np.concatenate([w_proj[sl, :], bp], axis=0)).astype(BF),
        })
    return in_maps


def kernel(x, w_attn, b_attn, w_proj, b_proj, _trace=False, _trace_kwargs=None):
    with_bias = bool(np.any(np.asarray(b_attn)) or np.any(np.asarray(b_proj)))
    key = ("nc", with_bias)
    if key not in _NC_CACHE:
        _NC_CACHE[key] = build_nc(with_bias=with_bias)
    nc = _NC_CACHE[key]
    in_maps = make_in_maps(x, w_attn, b_attn, w_proj, b_proj)
    kw = {}
    if _trace:
        kw["trace"] = True
        if _trace_kwargs:
            kw.update(_trace_kwargs)
    res = run_bass_kernel_spmd(nc, in_maps, core_ids=list(range(8)), **kw)
    outs = [res.results[c]["outT"] for c in range(8)]
    out = np.empty((B, T, D), dtype=np.float32)
    for b in range(B):
        np.add(outs[2 * b].T, outs[2 * b + 1].T, out=out[b])
    kernel._last_results = res
    return out


if __name__ == "__main__":
    nc = build_nc()
    fn = nc.m.functions[0]
    n = sum(len(blk.instructions) for blk in fn.blocks)
    print(f"built ok, static instructions: {n}")


# revision 6
# speedup vs baseline: 29.2495x; 1.2272x over previous
"""Causal self-attention Trainium2 Bass kernel — static-instruction-minimal.

Empirically this environment charges ~65us per STATIC instruction in the
NEFF (load/dispatch dominated), while executed instructions, matmul width,
and DMA descriptor counts are ~free.  The kernel therefore wraps all work
in dynamic For_i loops so the static program is ~400 instructions instead
of the ~2100 of a fully unrolled kernel.

Key devices to keep every loop-varying operand off the matmul stationary
port (walrus forbids register offsets in ldweights):
  - stationary tiles are staged into fixed scratch slots by DMA/DVE copies
    with dynamic source offsets;
  - PSUM accumulation groups are opened/closed by zero-stationary matmuls
    so start/stop flags stay static inside loops;
  - q/k and y are stored in head-major duplicated layouts ([64, slot, t])
    so all head/chunk addressing is affine in loop registers;
  - scores are computed full-width (no causal windowing — flops are free)
    and masked post-exp with a sliced static triangular mask;
  - the projection is computed transposed (out^T) so weights are the
    stationary; the host transposes back.

Sharding: 8 cores = (batch b in 0..3) x (head-group g in 0..1, 8 heads).
Core (b, g) computes a partial projection output for batch b over its 512
model dims; the host sums the two partials per batch (b_proj folded into
the g==0 core only).  Matmul inputs are shipped as bf16 (tolerance 2e-2).
"""

import numpy as np
import ml_dtypes

import concourse.bacc as bacc
import concourse.mybir as mybir
import concourse.tile as tile
from concourse.bass import ds
from concourse.bass_utils import run_bass_kernel_spmd

F32 = mybir.dt.float32
BF16 = mybir.dt.bfloat16
AF = mybir.ActivationFunctionType
BF = ml_dtypes.bfloat16

B, T, D, H = 4, 2048, 1024, 16
HD = 64              # head dim
HPC = 8              # heads per core
DC = HPC * HD        # 512 model dims per core
SCALE = 1.0 / np.sqrt(HD)

_NC_CACHE = {}


def build_nc(t=T, reps=1, with_bias=False):
    """Build the single-core SPMD program. t must be T for real runs.
    with_bias adds the ones x bias-row matmuls (spec fills biases with
    zeros, so the default program omits them)."""
    nt = t // 128
    KC = D // 128          # 8 contraction chunks over model dim

    nc = bacc.Bacc("TRN2", target_bir_lowering=False, debug=False)

    xT_d = nc.dram_tensor("xT", [D, t], BF16, kind="ExternalInput")
    # weight matrices carry the bias as an extra contraction row (vs ones)
    wqkv_d = nc.dram_tensor("wqkv", [D + 1, 3 * DC], BF16, kind="ExternalInput")
    wp_d = nc.dram_tensor("wp", [DC + 1, D], BF16, kind="ExternalInput")
    outT_d = nc.dram_tensor("outT", [D, t], F32, kind="ExternalOutput")

    NM = 3 * DC // 128     # 12 output chunks of 128 feats (q, k, v)

    with tile.TileContext(nc) as tc:
      for _rep in range(reps):
        with tc.tile_pool(name="ptop", bufs=1) as ptop:
          # normalized y^T, head-major: slot h rows 0:64 = head h dims
          ydup = ptop.tile([64, HPC, t], BF16)

          with tc.tile_pool(name="pab", bufs=1) as pab:
            # q/k/v head-major dup: slots 0-7 q^T, 8-15 k^T, 16-23 v^T
            qkvdup = pab.tile([64, 3 * HPC, t], BF16)
            # v natural, per head contiguous: vnat[p, h, tt, d] = v[tt*128+p, h, d]
            vnat = pab.tile([128, HPC, nt, HD], BF16, name="vnat", tag="vnat")
            # causal mask: Tm[p, u] = 1 iff u - t - p >= 0; slice
            # Tm[:, t - 128*kc :][:, :t] masks keys chunk kc vs all queries
            Tm = pab.tile([128, 2 * t], BF16, name="Tm", tag="Tm")
            onesr = pab.tile([1, t], BF16, name="onesr", tag="onesr")

            if with_bias:
                nc.gpsimd.memset(onesr[:], 1.0)
            nc.gpsimd.memset(Tm[:], 1.0)
            nc.gpsimd.affine_select(
                out=Tm[:], in_=Tm[:], compare_op=mybir.AluOpType.is_ge,
                fill=0.0, base=-t, pattern=[[1, 2 * t]], channel_multiplier=-1)

            # ---------------- Phase A: qkv ----------------
            with tc.tile_pool(name="pa", bufs=1) as pa, \
                 tc.tile_pool(name="pa_ps", bufs=1, space="PSUM") as paps:
                xTb = pa.tile([128, KC, t], BF16)
                ws = pa.tile([128, KC, 128], BF16, name="ws", tag="ws")
                wsb = pa.tile([1, 128], BF16, name="wsb", tag="wsb")
                nc.sync.dma_start(xTb[:], xT_d.rearrange("(k p) t -> p k t", p=128))

                psA = paps.tile([128, t], F32, name="psA", tag="psA")
                with tc.For_i(0, NM) as m:
                    nc.sync.dma_start(
                        ws[:], wqkv_d[0:D, ds(m * 128, 128)]
                        .rearrange("(k p) c -> p k c", p=128))
                    if with_bias:
                        nc.sync.dma_start(
                            wsb[:], wqkv_d[D:D + 1, ds(m * 128, 128)])
                    for k in range(KC):
                        for w in range(t // 512):
                            nc.tensor.matmul(
                                psA[:, w * 512:(w + 1) * 512],
                                ws[:, k, :], xTb[:, k, w * 512:(w + 1) * 512],
                                start=(k == 0),
                                stop=(not with_bias and k == KC - 1))
                    if with_bias:
                        for w in range(t // 512):
                            nc.tensor.matmul(
                                psA[:, w * 512:(w + 1) * 512],
                                wsb[:], onesr[:, w * 512:(w + 1) * 512],
                                start=False, stop=True)
                    nc.vector.tensor_copy(
                        qkvdup[:, ds(m * 2, 1), :], psA[0:64, :].unsqueeze(1))
                    nc.vector.tensor_copy(
                        qkvdup[:, ds(m * 2 + 1, 1), :], psA[64:128, :].unsqueeze(1))

                # v^T slots -> v natural via DMA transpose (contiguous dst;
                # HW writes dst[p, tt, d] = src[d, tt*128 + p])
                for h in range(HPC):
                    nc.sync.dma_start(
                        vnat[:, h, :, :], qkvdup[:, 2 * HPC + h, :],
                        transpose=True)

            # ---------------- Phase B: attention ----------------
            with tc.tile_pool(name="pb", bufs=1) as pb, \
                 tc.tile_pool(name="pb_ps", bufs=1, space="PSUM") as pbps:
                khs = pb.tile([64, 128], BF16, name="khs", tag="khs")
                # vhs col HD stays 1.0 (softmax denominator ones column)
                vhs = pb.tile([128, HD + 1], BF16, name="vhs", tag="vhs")
                nc.gpsimd.memset(vhs[:], 1.0)
                esb = pb.tile([128, t], BF16, name="esb", tag="esb")
                rec = pb.tile([1, t], F32, name="rec", tag="rec")
                rb = pb.tile([64, t], F32, name="rb", tag="rb")
                sp = pbps.tile([128, t], F32, name="sp", tag="sp")
                yacc = pbps.tile([HD + 1, t], F32, name="yacc", tag="yacc")
                with tc.For_i(0, HPC) as h:
                    nc.vector.memset(yacc[:], 0.0)
                    with tc.For_i(0, nt) as kc:
                        nc.vector.tensor_copy(
                            khs[:].unsqueeze(1),
                            qkvdup[:, ds(HPC + h, 1), ds(kc * 128, 128)])
                        for w in range(t // 512):
                            nc.tensor.matmul(
                                sp[:, w * 512:(w + 1) * 512],
                                khs[:],
                                qkvdup[:, ds(h, 1), w * 512:(w + 1) * 512],
                                start=True, stop=True)
                        nc.scalar.activation(esb[:], sp[:], AF.Exp,
                                             scale=float(SCALE))
                        nc.vector.tensor_mul(esb[:], esb[:],
                                             Tm[:, ds(t - kc * 128, t)])
                        nc.vector.tensor_copy(
                            vhs[:, 0:HD].unsqueeze(1),
                            vnat[:, ds(h, 1), ds(kc, 1), :]
                            .rearrange("p a b e -> p (a b) e"))
                        for w in range(t // 512):
                            nc.tensor.matmul(
                                yacc[:, w * 512:(w + 1) * 512],
                                vhs[:], esb[:, w * 512:(w + 1) * 512],
                                start=False, stop=False,
                                skip_group_check=True)
                    nc.vector.reciprocal(rec[:], yacc[HD:HD + 1, :])
                    nc.gpsimd.partition_broadcast(rb[:], rec[:])
                    nc.vector.tensor_mul(
                        ydup[:, ds(h, 1), :],
                        yacc[0:HD, :].unsqueeze(1), rb[:].unsqueeze(1))

          # ---------------- Phase C: projection (transposed) ----------------
          with tc.tile_pool(name="pc", bufs=1) as pc, \
               tc.tile_pool(name="pc_ps", bufs=1, space="PSUM") as pcps:
            osb = pc.tile([128, D // 128, t], F32, name="osb", tag="osb")
            wps = pc.tile([64, HPC, 128], BF16, name="wps", tag="wps")
            wpsb = pc.tile([1, 128], BF16, name="wpsb", tag="wpsb")
            onesc = pc.tile([1, t], BF16, name="onesc", tag="onesc")
            if with_bias:
                nc.gpsimd.memset(onesc[:], 1.0)
            pso = pcps.tile([128, t], F32, name="pso", tag="pso")
            with tc.For_i(0, D // 128) as oc:
                nc.sync.dma_start(
                    wps[:], wp_d[0:DC, ds(oc * 128, 128)]
                    .rearrange("(h p) c -> p h c", p=64))
                if with_bias:
                    nc.sync.dma_start(
                        wpsb[:], wp_d[DC:DC + 1, ds(oc * 128, 128)])
                for h in range(HPC):
                    for w in range(t // 512):
                        nc.tensor.matmul(
                            pso[:, w * 512:(w + 1) * 512],
                            wps[:, h, :], ydup[:, h, w * 512:(w + 1) * 512],
                            start=(h == 0),
                            stop=(not with_bias and h == HPC - 1))
                if with_bias:
                    for w in range(t // 512):
                        nc.tensor.matmul(
                            pso[:, w * 512:(w + 1) * 512],
                            wpsb[:], onesc[:, w * 512:(w + 1) * 512],
                            start=False, stop=True)
                nc.vector.tensor_copy(
                    osb[:, ds(oc, 1), :], pso[:].unsqueeze(1))
            nc.sync.dma_start(outT_d.rearrange("(c p) t -> p c t", p=128), osb[:])

    nc.finalize()
    return nc


def make_in_maps(x, w_attn, b_attn, w_proj, b_proj):
    x = np.asarray(x, dtype=np.float32)
    w_attn = np.asarray(w_attn, dtype=np.float32)
    b_attn = np.asarray(b_attn, dtype=np.float32)
    w_proj = np.asarray(w_proj, dtype=np.float32)
    b_proj = np.asarray(b_proj, dtype=np.float32)
    in_maps = []
    for c in range(8):
        b, g = c // 2, c % 2
        sl = slice(DC * g, DC * (g + 1))
        wqkv = np.concatenate(
            [w_attn[:, 0 * D:1 * D][:, sl],
             w_attn[:, 1 * D:2 * D][:, sl],
             w_attn[:, 2 * D:3 * D][:, sl]], axis=1)
        bqkv = np.concatenate(
            [b_attn[0 * D:1 * D][sl], b_attn[1 * D:2 * D][sl],
             b_attn[2 * D:3 * D][sl]])[None, :]
        bp = (b_proj if g == 0 else np.zeros_like(b_proj))[None, :]
        in_maps.append({
            "xT": np.ascontiguousarray(x[b].T).astype(BF),
            "wqkv": np.ascontiguousarray(
                np.concatenate([wqkv, bqkv], axis=0)).astype(BF),
            "wp": np.ascontiguousarray(
                np.concatenate([w_proj[sl, :], bp], axis=0)).astype(BF),
        })
    return in_maps


def kernel(x, w_attn, b_attn, w_proj, b_proj, _trace=False, _trace_kwargs=None):
    with_bias = bool(np.any(np.asarray(b_attn)) or np.any(np.asarray(b_proj)))
    key = ("nc", with_bias)
    if key not in _NC_CACHE:
        _NC_CACHE[key] = build_nc(with_bias=with_bias)
    nc = _NC_CACHE[key]
    in_maps = make_in_maps(x, w_attn, b_attn, w_proj, b_proj)
    kw = {}
    if _trace:
        kw["trace"] = True
        if _trace_kwargs:
            kw.update(_trace_kwargs)
    res = run_bass_kernel_spmd(nc, in_maps, core_ids=list(range(8)), **kw)
    outs = [res.results[c]["outT"] for c in range(8)]
    out = np.empty((B, T, D), dtype=np.float32)
    for b in range(B):
        np.add(outs[2 * b].T, outs[2 * b + 1].T, out=out[b])
    kernel._last_results = res
    return out


if __name__ == "__main__":
    nc = build_nc()
    fn = nc.m.functions[0]
    n = sum(len(blk.instructions) for blk in fn.blocks)
    print(f"built ok, static instructions: {n}")
